# revision 3
# baseline (speedup 1.0000x reference)
"""CMHSA Trainium2 kernel v3 (nn_CMHSA_56487409877161).

v3 structure per core (4 batches):
  startconv fwd only (splain, spos; spos emits qsum via accum_out).
  Per (b,h): qwlt = splain_h^T @ W_lin'  (8 J=64 matmuls, [128,8,64] bf16),
  ET matmuls (softmax axis on partitions), exp on ACT, square on DVE,
  then per chunk: H (=W_lin G fused, lhsT=qwlt chunk), r/ssq eyes streams.
  Batch stats as in v2. Phase B: cbc pair broadcast (2 K=1 matmuls into one
  [128,N] psum), t1 = H_all*cbc (PSUM operand), f = t1 + beta + x, pair-wide.
"""

import numpy as np

import concourse.bass as bass
import concourse.mybir as mybir
import concourse.tile as tile
from concourse import bacc, bass_utils

B, C, N = 32, 512, 1024
HEADS, DH = 8, 64
NCORES = 8
BPC = B // NCORES
EPS = 1e-5
SCALE = (C / 4.0) ** 0.5
SQ = float(np.sqrt(SCALE))
EBIAS = 45.0
MU = 1.0 / N
BUILD_SALT = 35

F32 = mybir.dt.float32
BF16 = mybir.dt.bfloat16
AF = mybir.ActivationFunctionType
ALU = mybir.AluOpType

MMDT = BF16


def build_program():
    nc = bacc.Bacc("TRN2", target_bir_lowering=False)
    dt = F32
    pdt = BF16
    xin = nc.dram_tensor("xin", [BPC, C, N], MMDT, kind="ExternalInput").ap()
    xrd = nc.dram_tensor("xrd", [BPC, C, N], F32, kind="ExternalInput").ap()
    wco = nc.dram_tensor("wco", [C, C], MMDT, kind="ExternalInput").ap()
    posd = nc.dram_tensor("posd", [C, N], dt, kind="ExternalInput").ap()
    bc128 = nc.dram_tensor("bc128", [128, 4], dt, kind="ExternalInput").ap()
    qcorr = nc.dram_tensor("qcorr", [128, 4], dt, kind="ExternalInput").ap()
    wl = nc.dram_tensor("wl", [128, DH], F32, kind="ExternalInput").ap()
    wlb = nc.dram_tensor("wlb", [128, DH], MMDT, kind="ExternalInput").ap()
    blin2 = nc.dram_tensor("blin2", [128, 1], dt, kind="ExternalInput").ap()
    eyesd = nc.dram_tensor("eyesd", [128, HEADS, HEADS], pdt, kind="ExternalInput").ap()
    oner = nc.dram_tensor("oner", [1, 128], F32, kind="ExternalInput").ap()
    sel2 = nc.dram_tensor("sel2", [2, 128], F32, kind="ExternalInput").ap()
    cbias = nc.dram_tensor("cbias", [128, 3 + BUILD_SALT], dt, kind="ExternalInput").ap()
    outd = nc.dram_tensor("outd", [BPC, C, N], dt, kind="ExternalOutput").ap()

    act = nc.scalar
    vec = nc.vector
    pe = nc.tensor

    with tile.TileContext(nc) as tc:
        with (
            tc.tile_pool(name="consts", bufs=1) as consts,
            tc.tile_pool(name="xpool", bufs=1) as xpool,
            tc.tile_pool(name="spool", bufs=1) as spool,
            tc.tile_pool(name="gpool", bufs=1) as gpool,
            tc.tile_pool(name="qwpool", bufs=3) as qwpool,
            tc.tile_pool(name="ppool", bufs=3) as ppool,
            tc.tile_pool(name="p2pool", bufs=2) as p2pool,
            tc.tile_pool(name="tails", bufs=2) as tails,
            tc.tile_pool(name="stats", bufs=1) as stats,
            tc.tile_pool(name="psum", bufs=1, space="PSUM") as psum,
        ):
            wco_sb = consts.tile([128, 4, C], MMDT)
            nc.sync.dma_start(wco_sb[:], wco.rearrange("(cc p) o -> p cc o", p=128))
            pos_sb = consts.tile([128, 4, N], dt)
            nc.sync.dma_start(pos_sb[:], posd.rearrange("(cc p) n -> p cc n", p=128))
            bc128_sb = consts.tile([128, 4], dt)
            nc.sync.dma_start(bc128_sb[:], bc128)
            qcorr_sb = consts.tile([128, 4], dt)
            nc.sync.dma_start(qcorr_sb[:], qcorr)
            wl_sb = consts.tile([128, DH], F32)
            nc.sync.dma_start(wl_sb[:], wl)
            wlb_sb = consts.tile([128, DH], MMDT)
            nc.sync.dma_start(wlb_sb[:], wlb)
            blin2_sb = consts.tile([128, 1], dt)
            nc.sync.dma_start(blin2_sb[:], blin2)
            eyes_sb = consts.tile([128, HEADS, HEADS], pdt)
            nc.sync.dma_start(eyes_sb[:], eyesd)
            oner_sb = consts.tile([1, 128], F32)
            nc.sync.dma_start(oner_sb[:], oner)
            sel2_sb = consts.tile([2, 128], F32)
            nc.sync.dma_start(sel2_sb[:], sel2)
            cb_sb = consts.tile([128, 3], dt)
            nc.sync.dma_start(cb_sb[:], cbias[:, 0:3])
            zeros_sb = consts.tile([128, N], dt)
            nc.vector.memset(zeros_sb[:], 0.0)

            for b in range(BPC):
                # ---------- startconv forward
                x_sb = xpool.tile([128, 4, N], MMDT, tag="x", name=f"x_{b}")
                nc.sync.dma_start(x_sb[:], xin[b].rearrange("(cc p) n -> p cc n", p=128))

                splain = spool.tile([128, 4, N], MMDT, tag="splain", name=f"splain_{b}")
                spos = spool.tile([128, 4, N], MMDT, tag="spos", name=f"spos_{b}")
                qs_raw = stats.tile([128, 4], dt, tag="qs_raw", name=f"qsraw_{b}")

                for pc in range(4):
                    s_ps = psum.tile([128, N], dt, tag="et", bufs=2, name=f"sps_{b}_{pc}")
                    for cc in range(4):
                        for half in range(2):
                            pe.matmul(
                                s_ps[:, half * 512 : half * 512 + 512],
                                lhsT=wco_sb[:, cc, 128 * pc : 128 * pc + 128],
                                rhs=x_sb[:, cc, half * 512 : half * 512 + 512],
                                start=(cc == 0),
                                stop=(cc == 3),
                            )
                    vec.scalar_tensor_tensor(
                        out=splain[:, pc, :], in0=s_ps[:],
                        scalar=bc128_sb[:, pc : pc + 1],
                        in1=zeros_sb[:], op0=ALU.add, op1=ALU.add,
                        accum_out=qs_raw[:, pc : pc + 1],
                    )
                    vec.scalar_tensor_tensor(
                        out=spos[:, pc, :], in0=s_ps[:],
                        scalar=bc128_sb[:, pc : pc + 1],
                        in1=pos_sb[:, pc, :], op0=ALU.add, op1=ALU.add,
                    )

                # qs_used = qs_raw - qcorr  (= qsum/SQ in channel layout)
                qs_used = stats.tile([128, 4], dt, tag="qs_used", name=f"qsu_{b}")
                vec.tensor_tensor(qs_used[:], qs_raw[:], qcorr_sb[:], ALU.subtract)

                # wq[e, h] = sum_d wl[d, e] * qs_used[d_h]  (per head)
                wq_ps = psum.tile([DH, HEADS], dt, tag="g", bufs=1, name=f"wqps_{b}")
                for h in range(HEADS):
                    prow = (h % 2) * 64
                    pcix = h // 2
                    pe.matmul(
                        wq_ps[:, h : h + 1],
                        lhsT=wl_sb[prow : prow + 64, :],
                        rhs=qs_used[prow : prow + 64, pcix : pcix + 1],
                        start=True, stop=True,
                        tile_position=(prow, 0),
                    )
                wq_sb = stats.tile([DH, HEADS], dt, tag="wq", name=f"wq_{b}")
                vec.tensor_copy(wq_sb[:], wq_ps[:])

                # ---------- per-head maps
                rs_ps = psum.tile([104, N], dt, tag="rs", name=f"rsps_{b}")
                g_all = gpool.tile([128, 4, N], F32, tag="g_all", name=f"gall_{b}")

                for px in range(4):
                    h0, h1 = 2 * px, 2 * px + 1
                    # qwlt[n, e] = sum_d splain_h[d, n] * wlb[d, e]  (both heads)
                    qw_ps = psum.tile([128, 8 * DH], dt, tag="et", bufs=2, name=f"qwps_{b}_{h0}")
                    qw_ps2 = psum.tile([128, 8 * DH], dt, tag="et", bufs=2, name=f"qwps_{b}_{h1}")
                    for sc in range(8):
                        pe.matmul(
                            qw_ps[:, DH * sc : DH * sc + DH],
                            lhsT=splain[0:64, px, 128 * sc : 128 * sc + 128],
                            rhs=wlb_sb[0:64, :],
                            start=True, stop=True,
                            tile_position=(0, 0),
                        )
                        pe.matmul(
                            qw_ps2[:, DH * sc : DH * sc + DH],
                            lhsT=splain[64:128, px, 128 * sc : 128 * sc + 128],
                            rhs=wlb_sb[64:128, :],
                            start=True, stop=True,
                            tile_position=(64, 0),
                        )
                    qwlt0 = qwpool.tile([128, 8, DH], MMDT, tag="qwlt", name=f"qwlt_{b}_{h0}")
                    vec.tensor_copy(qwlt0[:], qw_ps[:])
                    qwlt1 = qwpool.tile([128, 8, DH], MMDT, tag="qwlt", name=f"qwlt_{b}_{h1}")
                    vec.tensor_copy(qwlt1[:], qw_ps2[:])

                    g_pair = psum.tile([128, N], dt, tag="g", bufs=1, name=f"gps_{b}_{px}")
                    for sc in range(8):
                        et0 = psum.tile([128, N], dt, tag="et", bufs=2, name=f"et_{b}_{h0}_{sc}")
                        et1 = psum.tile([128, N], dt, tag="et", bufs=2, name=f"et_{b}_{h1}_{sc}")
                        for half in range(2):
                            sl = slice(half * 512, half * 512 + 512)
                            pe.matmul(
                                et0[:, sl],
                                lhsT=splain[0:64, px, 128 * sc : 128 * sc + 128],
                                rhs=spos[0:64, px, sl],
                                start=True, stop=True,
                                tile_position=(0, 0),
                            )
                            pe.matmul(
                                et1[:, sl],
                                lhsT=splain[64:128, px, 128 * sc : 128 * sc + 128],
                                rhs=spos[64:128, px, sl],
                                start=True, stop=True,
                                tile_position=(64, 0),
                            )
                        p0 = ppool.tile([128, N], pdt, tag="p", name=f"p_{b}_{h0}_{sc}")
                        act.activation(p0[:], et0[:], AF.Exp, bias=cb_sb[:, 0:1], scale=1.0)
                        p1 = ppool.tile([128, N], pdt, tag="p", name=f"p_{b}_{h1}_{sc}")
                        act.activation(p1[:], et1[:], AF.Exp, bias=cb_sb[:, 0:1], scale=1.0)
                        p20 = p2pool.tile([128, N], pdt, tag="p2", name=f"p2_{b}_{h0}_{sc}")
                        vec.tensor_tensor(p20[:], p0[:], p0[:], ALU.mult)
                        p21 = p2pool.tile([128, N], pdt, tag="p2", name=f"p2_{b}_{h1}_{sc}")
                        vec.tensor_tensor(p21[:], p1[:], p1[:], ALU.mult)
                        for half in range(2):
                            sl = slice(half * 512, half * 512 + 512)
                            pe.matmul(
                                g_pair[0:64, sl], lhsT=qwlt0[:, sc, :], rhs=p0[:, sl],
                                start=(sc == 0), stop=(sc == 7),
                                tile_position=(0, 0),
                            )
                            pe.matmul(
                                g_pair[64:128, sl], lhsT=qwlt1[:, sc, :], rhs=p1[:, sl],
                                start=(sc == 0), stop=(sc == 7),
                                tile_position=(0, 64),
                            )
                            pe.matmul(
                                rs_ps[64:72, sl], lhsT=eyes_sb[:, h0, :], rhs=p0[:, sl],
                                start=(px == 0 and sc == 0), stop=False,
                                tile_position=(0, 64),
                            )
                            pe.matmul(
                                rs_ps[64:72, sl], lhsT=eyes_sb[:, h1, :], rhs=p1[:, sl],
                                start=False, stop=(px == 3 and sc == 7),
                                tile_position=(0, 64),
                            )
                            pe.matmul(
                                rs_ps[96:104, sl], lhsT=eyes_sb[:, h0, :], rhs=p20[:, sl],
                                start=(px == 0 and sc == 0), stop=False,
                                tile_position=(0, 96),
                            )
                            pe.matmul(
                                rs_ps[96:104, sl], lhsT=eyes_sb[:, h1, :], rhs=p21[:, sl],
                                start=False, stop=(px == 3 and sc == 7),
                                tile_position=(0, 96),
                            )
                    act.activation(g_all[:, px, :], g_pair[:], AF.Copy)

                # ---------- batch stats (unchanged from v2)
                rs_stage = stats.tile([104, N], dt, tag="rs_stage", name=f"rsst_{b}")
                vec.tensor_copy(rs_stage[64:72, :], rs_ps[64:72, :])
                vec.tensor_copy(rs_stage[96:104, :], rs_ps[96:104, :])
                r_b = stats.tile([HEADS, N], dt, tag="r_b", name=f"rb_{b}")
                nc.sync.dma_start(r_b[:], rs_stage[64:72, :])
                ssq_sb = stats.tile([HEADS, N], dt, tag="ssq_sb", name=f"ssqsb_{b}")
                nc.sync.dma_start(ssq_sb[:], rs_stage[96:104, :])
                rinv = stats.tile([HEADS, N], dt, tag="rinv", name=f"rinv_{b}")
                vec.reciprocal(rinv[:], r_b[:])
                rinvsq = stats.tile([HEADS, N], dt, tag="rinvsq", name=f"rinvsq_{b}")
                vec.tensor_tensor(rinvsq[:], rinv[:], rinv[:], ALU.mult)
                ttr_scr = stats.tile([HEADS, N], dt, tag="ttr", name=f"ttr_{b}")
                vec.tensor_tensor(ttr_scr[:], ssq_sb[:], rinvsq[:], ALU.mult)
                s2 = stats.tile([HEADS, 1], dt, tag="s2", name=f"s2_{b}")
                vec.reduce_sum(s2[:], ttr_scr[:], axis=mybir.AxisListType.X)
                var = stats.tile([HEADS, 1], dt, tag="var", name=f"var_{b}")
                vec.tensor_scalar(
                    out=var[:], in0=s2[:], scalar1=1.0 / (float(N) * float(N)),
                    scalar2=-MU * MU, op0=ALU.mult, op1=ALU.add,
                )
                lnv = stats.tile([HEADS, 1], dt, tag="lnv", name=f"lnv_{b}")
                act.activation(lnv[:], var[:], AF.Ln, bias=cb_sb[0:HEADS, 1:2], scale=1.0)
                istd = stats.tile([HEADS, 1], dt, tag="istd", name=f"istd_{b}")
                act.activation(istd[:], lnv[:], AF.Exp, bias=cb_sb[0:HEADS, 2:3], scale=-0.5)
                c_b = stats.tile([HEADS, N], dt, tag="c_b", name=f"cb_{b}")
                vec.tensor_scalar(
                    out=c_b[:], in0=rinv[:], scalar1=istd[:], scalar2=None, op0=ALU.mult
                )
                istd_t = stats.tile([1, HEADS], dt, tag="istd_t", name=f"istdt_{b}")
                for h in range(HEADS):
                    nc.sync.dma_start(istd_t[0:1, h : h + 1], istd[h : h + 1, 0:1])
                ibc_ps = psum.tile([DH, HEADS], dt, tag="g", bufs=1, name=f"ibcps_{b}")
                pe.matmul(ibc_ps[:], lhsT=oner_sb[0:1, 0:DH], rhs=istd_t[:], start=True, stop=True)
                istd_bc = stats.tile([DH, HEADS], dt, tag="istd_bc", name=f"istdbc_{b}")
                act.activation(istd_bc[:], ibc_ps[:], AF.Copy)
                beta_t = stats.tile([DH, HEADS], dt, tag="beta_t", name=f"betat_{b}")
                vec.scalar_tensor_tensor(
                    out=beta_t[:], in0=wq_sb[:], scalar=-MU, in1=istd_bc[:],
                    op0=ALU.mult, op1=ALU.mult,
                )
                beta_sb = stats.tile([DH, HEADS], dt, tag="beta", name=f"beta_{b}")
                vec.tensor_scalar(
                    out=beta_sb[:], in0=beta_t[:], scalar1=blin2_sb[0:DH, :], scalar2=None,
                    op0=ALU.add,
                )

                # ---------- Phase B: pair-wide tail (no W_lin matmul needed)
                for px in range(4):
                    h = 2 * px
                    c_row = tails.tile([2, N], dt, tag="c_row", name=f"crow_{b}_{px}")
                    nc.sync.dma_start(c_row[0:1, :], c_b[h : h + 1, :])
                    nc.sync.dma_start(c_row[1:2, :], c_b[h + 1 : h + 2, :])
                    cbc_ps = psum.tile([128, N], dt, tag="et", bufs=2, name=f"cbcps_{b}_{px}")
                    for half in range(2):
                        sl = slice(half * 512, half * 512 + 512)
                        pe.matmul(
                            cbc_ps[:, sl], lhsT=sel2_sb[:], rhs=c_row[:, sl],
                            start=True, stop=True,
                        )
                    beta_pair = tails.tile([128, 1], dt, tag="beta_pair", name=f"bp_{b}_{px}")
                    nc.sync.dma_start(beta_pair[0:64, :], beta_sb[:, h : h + 1])
                    nc.sync.dma_start(beta_pair[64:128, :], beta_sb[:, h + 1 : h + 2])
                    xres = tails.tile([128, N], dt, tag="xres", name=f"xres_{b}_{px}")
                    nc.sync.dma_start(xres[:], xrd[b, 128 * px : 128 * px + 128, :])
                    t1 = tails.tile([128, N], dt, tag="t1", name=f"t1_{b}_{px}")
                    vec.tensor_tensor(t1[:], g_all[:, px, :], cbc_ps[:], ALU.mult)
                    f_sb = tails.tile([128, N], dt, tag="f", name=f"f_{b}_{px}")
                    vec.scalar_tensor_tensor(
                        out=f_sb[:], in0=t1[:], scalar=beta_pair[:],
                        in1=xres[:], op0=ALU.add, op1=ALU.add,
                    )
                    nc.sync.dma_start(outd[b, 128 * px : 128 * px + 128, :], f_sb[:])

    nc.compile()
    return nc


def host_inputs(x, W_start, b_start, rel_h, rel_w, W_lin, b_lin):
    x = np.asarray(x, np.float32)
    W_start = np.asarray(W_start, np.float32)
    b_start = np.asarray(b_start, np.float32)
    pos = (np.asarray(rel_h, np.float32) + np.asarray(rel_w, np.float32)).reshape(
        HEADS, DH, N
    )
    W_lin = np.asarray(W_lin, np.float32)
    b_lin = np.asarray(b_lin, np.float32)
    import ml_dtypes

    bf = ml_dtypes.bfloat16
    posd = np.ascontiguousarray((pos * SQ).reshape(C, N).astype(np.float32))
    bc = (b_start / SQ).reshape(4, 128).T.astype(np.float32)
    qc = bc * float(N)
    wlT = np.concatenate([(W_lin * SQ).T, (W_lin * SQ).T], axis=0)
    consts = {
        "wco": np.ascontiguousarray((W_start.T / SQ).astype(bf)),
        "posd": posd,
        "bc128": np.ascontiguousarray(bc),
        "qcorr": np.ascontiguousarray(qc.astype(np.float32)),
        "wl": np.ascontiguousarray(wlT.astype(np.float32)),
        "wlb": np.ascontiguousarray(wlT.astype(bf)),
        "blin2": np.ascontiguousarray(np.tile(b_lin, 2)[:, None].astype(np.float32)),
        "eyesd": np.ascontiguousarray(
            np.broadcast_to(np.eye(HEADS, dtype=np.float32), (128, HEADS, HEADS))
        ).astype(bf),
        "oner": np.ones((1, 128), np.float32),
        "sel2": np.ascontiguousarray(
            np.stack([
                np.concatenate([np.ones(64), np.zeros(64)]),
                np.concatenate([np.zeros(64), np.ones(64)]),
            ]).astype(np.float32)
        ),
        "cbias": np.ascontiguousarray(
            np.broadcast_to(
                np.array([-EBIAS, EPS, 0.0] + [0.0] * BUILD_SALT, np.float32),
                (128, 3 + BUILD_SALT),
            )
        ),
    }
    xr = x.reshape(B, C, N)
    in_maps = []
    for c in range(NCORES):
        m = dict(consts)
        m["xin"] = np.ascontiguousarray(xr[c * BPC : (c + 1) * BPC].astype(bf))
        m["xrd"] = np.ascontiguousarray(xr[c * BPC : (c + 1) * BPC])
        in_maps.append(m)
    return in_maps


_PROG = None


def kernel(**inputs):
    global _PROG
    if _PROG is None:
        _PROG = build_program()
    in_maps = host_inputs(**inputs)
    res = bass_utils.run_bass_kernel_spmd(_PROG, in_maps, core_ids=list(range(NCORES)))
    out = np.concatenate([r["outd"] for r in res.results], axis=0)
    return out.reshape(B, C, 32, 32)


# revision 4
# speedup vs baseline: 1.0970x; 1.0970x over previous
"""CMHSA Trainium2 kernel v3 (nn_CMHSA_56487409877161).

v3 structure per core (4 batches):
  startconv fwd only (splain, spos; spos emits qsum via accum_out).
  Per (b,h): qwlt = splain_h^T @ W_lin'  (8 J=64 matmuls, [128,8,64] bf16),
  ET matmuls (softmax axis on partitions), exp on ACT, square on DVE,
  then per chunk: H (=W_lin G fused, lhsT=qwlt chunk), r/ssq eyes streams.
  Batch stats as in v2. Phase B: cbc pair broadcast (2 K=1 matmuls into one
  [128,N] psum), t1 = H_all*cbc (PSUM operand), f = t1 + beta + x, pair-wide.
"""

import numpy as np

import concourse.bass as bass
import concourse.mybir as mybir
import concourse.tile as tile
from concourse import bacc, bass_utils

B, C, N = 32, 512, 1024
HEADS, DH = 8, 64
NCORES = 8
BPC = B // NCORES
EPS = 1e-5
SCALE = (C / 4.0) ** 0.5
SQ = float(np.sqrt(SCALE))
EBIAS = 45.0
MU = 1.0 / N
BUILD_SALT = 37

F32 = mybir.dt.float32
BF16 = mybir.dt.bfloat16
AF = mybir.ActivationFunctionType
ALU = mybir.AluOpType

MMDT = BF16


def build_program():
    nc = bacc.Bacc("TRN2", target_bir_lowering=False)
    dt = F32
    pdt = BF16
    xin = nc.dram_tensor("xin", [BPC, C, N], MMDT, kind="ExternalInput").ap()
    xrd = nc.dram_tensor("xrd", [BPC, C, N], F32, kind="ExternalInput").ap()
    wco = nc.dram_tensor("wco", [C, C], MMDT, kind="ExternalInput").ap()
    posd = nc.dram_tensor("posd", [C, N], dt, kind="ExternalInput").ap()
    bc128 = nc.dram_tensor("bc128", [128, 4], dt, kind="ExternalInput").ap()
    qcorr = nc.dram_tensor("qcorr", [128, 4], dt, kind="ExternalInput").ap()
    wl = nc.dram_tensor("wl", [128, DH], F32, kind="ExternalInput").ap()
    wlb = nc.dram_tensor("wlb", [128, DH], MMDT, kind="ExternalInput").ap()
    blin2 = nc.dram_tensor("blin2", [128, 1], dt, kind="ExternalInput").ap()
    eyesd = nc.dram_tensor("eyesd", [128, HEADS, HEADS], pdt, kind="ExternalInput").ap()
    oner = nc.dram_tensor("oner", [1, 128], F32, kind="ExternalInput").ap()
    sel2 = nc.dram_tensor("sel2", [2, 128], F32, kind="ExternalInput").ap()
    cbias = nc.dram_tensor("cbias", [128, 3 + BUILD_SALT], dt, kind="ExternalInput").ap()
    outd = nc.dram_tensor("outd", [BPC, C, N], dt, kind="ExternalOutput").ap()

    act = nc.scalar
    vec = nc.vector
    pe = nc.tensor

    with tile.TileContext(nc) as tc:
        with (
            tc.tile_pool(name="consts", bufs=1) as consts,
            tc.tile_pool(name="xpool", bufs=1) as xpool,
            tc.tile_pool(name="spool", bufs=1) as spool,
            tc.tile_pool(name="gpool", bufs=1) as gpool,
            tc.tile_pool(name="qwpool", bufs=3) as qwpool,
            tc.tile_pool(name="ppool", bufs=3) as ppool,
            tc.tile_pool(name="p2pool", bufs=2) as p2pool,
            tc.tile_pool(name="tails", bufs=2) as tails,
            tc.tile_pool(name="stats", bufs=1) as stats,
            tc.tile_pool(name="psum", bufs=1, space="PSUM") as psum,
        ):
            wco_sb = consts.tile([128, 4, C], MMDT)
            nc.sync.dma_start(wco_sb[:], wco.rearrange("(cc p) o -> p cc o", p=128))
            pos_sb = consts.tile([128, 4, N], dt)
            nc.sync.dma_start(pos_sb[:], posd.rearrange("(cc p) n -> p cc n", p=128))
            bc128_sb = consts.tile([128, 4], dt)
            nc.sync.dma_start(bc128_sb[:], bc128)
            qcorr_sb = consts.tile([128, 4], dt)
            nc.sync.dma_start(qcorr_sb[:], qcorr)
            wl_sb = consts.tile([128, DH], F32)
            nc.sync.dma_start(wl_sb[:], wl)
            wlb_sb = consts.tile([128, DH], MMDT)
            nc.sync.dma_start(wlb_sb[:], wlb)
            blin2_sb = consts.tile([128, 1], dt)
            nc.sync.dma_start(blin2_sb[:], blin2)
            eyes_sb = consts.tile([128, HEADS, HEADS], pdt)
            nc.sync.dma_start(eyes_sb[:], eyesd)
            oner_sb = consts.tile([1, 128], F32)
            nc.sync.dma_start(oner_sb[:], oner)
            sel2_sb = consts.tile([2, 128], F32)
            nc.sync.dma_start(sel2_sb[:], sel2)
            cb_sb = consts.tile([128, 3], dt)
            nc.sync.dma_start(cb_sb[:], cbias[:, 0:3])
            zeros_sb = consts.tile([128, N], dt)
            nc.vector.memset(zeros_sb[:], 0.0)

            for b in range(BPC):
                # ---------- startconv forward
                x_sb = xpool.tile([128, 4, N], MMDT, tag="x", name=f"x_{b}")
                nc.sync.dma_start(x_sb[:], xin[b].rearrange("(cc p) n -> p cc n", p=128))

                splain = spool.tile([128, 4, N], MMDT, tag="splain", name=f"splain_{b}")
                spos = spool.tile([128, 4, N], MMDT, tag="spos", name=f"spos_{b}")
                qs_raw = stats.tile([128, 4], dt, tag="qs_raw", name=f"qsraw_{b}")

                for pc in range(4):
                    s_ps = psum.tile([128, N], dt, tag="et", bufs=2, name=f"sps_{b}_{pc}")
                    for cc in range(4):
                        for half in range(2):
                            pe.matmul(
                                s_ps[:, half * 512 : half * 512 + 512],
                                lhsT=wco_sb[:, cc, 128 * pc : 128 * pc + 128],
                                rhs=x_sb[:, cc, half * 512 : half * 512 + 512],
                                start=(cc == 0),
                                stop=(cc == 3),
                            )
                    vec.scalar_tensor_tensor(
                        out=splain[:, pc, :], in0=s_ps[:],
                        scalar=bc128_sb[:, pc : pc + 1],
                        in1=zeros_sb[:], op0=ALU.add, op1=ALU.add,
                        accum_out=qs_raw[:, pc : pc + 1],
                    )
                    vec.scalar_tensor_tensor(
                        out=spos[:, pc, :], in0=s_ps[:],
                        scalar=bc128_sb[:, pc : pc + 1],
                        in1=pos_sb[:, pc, :], op0=ALU.add, op1=ALU.add,
                    )

                # qs_used = qs_raw - qcorr  (= qsum/SQ in channel layout)
                qs_used = stats.tile([128, 4], dt, tag="qs_used", name=f"qsu_{b}")
                vec.tensor_tensor(qs_used[:], qs_raw[:], qcorr_sb[:], ALU.subtract)

                # wq[e, h] = sum_d wl[d, e] * qs_used[d_h]  (per head)
                wq_ps = psum.tile([DH, HEADS], dt, tag="g", bufs=1, name=f"wqps_{b}")
                for h in range(HEADS):
                    prow = (h % 2) * 64
                    pcix = h // 2
                    pe.matmul(
                        wq_ps[:, h : h + 1],
                        lhsT=wl_sb[prow : prow + 64, :],
                        rhs=qs_used[prow : prow + 64, pcix : pcix + 1],
                        start=True, stop=True,
                        tile_position=(prow, 0),
                    )
                wq_sb = stats.tile([DH, HEADS], dt, tag="wq", name=f"wq_{b}")
                vec.tensor_copy(wq_sb[:], wq_ps[:])

                # ---------- per-head maps
                rs_ps = psum.tile([104, N], dt, tag="rs", name=f"rsps_{b}")
                g_all = gpool.tile([128, 4, N], F32, tag="g_all", name=f"gall_{b}")

                for px in range(4):
                    h0, h1 = 2 * px, 2 * px + 1
                    # qwlt[n, e] = sum_d splain_h[d, n] * wlb[d, e]  (both heads)
                    qw_ps = psum.tile([128, 8 * DH], dt, tag="et", bufs=2, name=f"qwps_{b}_{h0}")
                    qw_ps2 = psum.tile([128, 8 * DH], dt, tag="et", bufs=2, name=f"qwps_{b}_{h1}")
                    for sc in range(8):
                        pe.matmul(
                            qw_ps[:, DH * sc : DH * sc + DH],
                            lhsT=splain[0:64, px, 128 * sc : 128 * sc + 128],
                            rhs=wlb_sb[0:64, :],
                            start=True, stop=True,
                            tile_position=(0, 0),
                        )
                        pe.matmul(
                            qw_ps2[:, DH * sc : DH * sc + DH],
                            lhsT=splain[64:128, px, 128 * sc : 128 * sc + 128],
                            rhs=wlb_sb[64:128, :],
                            start=True, stop=True,
                            tile_position=(64, 0),
                        )
                    qwlt0 = qwpool.tile([128, 8, DH], MMDT, tag="qwlt", name=f"qwlt_{b}_{h0}")
                    vec.tensor_copy(qwlt0[:], qw_ps[:])
                    qwlt1 = qwpool.tile([128, 8, DH], MMDT, tag="qwlt", name=f"qwlt_{b}_{h1}")
                    vec.tensor_copy(qwlt1[:], qw_ps2[:])

                    g_pair = psum.tile([128, N], dt, tag="g", bufs=1, name=f"gps_{b}_{px}")
                    for sc in range(8):
                        et0 = psum.tile([128, N], dt, tag="et", bufs=2, name=f"et_{b}_{h0}_{sc}")
                        et1 = psum.tile([128, N], dt, tag="et", bufs=2, name=f"et_{b}_{h1}_{sc}")
                        for half in range(2):
                            sl = slice(half * 512, half * 512 + 512)
                            pe.matmul(
                                et0[:, sl],
                                lhsT=splain[0:64, px, 128 * sc : 128 * sc + 128],
                                rhs=spos[0:64, px, sl],
                                start=True, stop=True,
                                tile_position=(0, 0),
                            )
                            pe.matmul(
                                et1[:, sl],
                                lhsT=splain[64:128, px, 128 * sc : 128 * sc + 128],
                                rhs=spos[64:128, px, sl],
                                start=True, stop=True,
                                tile_position=(64, 0),
                            )
                        p0 = ppool.tile([128, N], pdt, tag="p", name=f"p_{b}_{h0}_{sc}")
                        act.activation(p0[:], et0[:], AF.Exp, bias=cb_sb[:, 0:1], scale=1.0)
                        p1 = ppool.tile([128, N], pdt, tag="p", name=f"p_{b}_{h1}_{sc}")
                        act.activation(p1[:], et1[:], AF.Exp, bias=cb_sb[:, 0:1], scale=1.0)
                        p20 = p2pool.tile([128, N], pdt, tag="p2", name=f"p2_{b}_{h0}_{sc}")
                        vec.tensor_tensor(p20[:], p0[:], p0[:], ALU.mult)
                        p21 = p2pool.tile([128, N], pdt, tag="p2", name=f"p2_{b}_{h1}_{sc}")
                        vec.tensor_tensor(p21[:], p1[:], p1[:], ALU.mult)
                        for half in range(2):
                            sl = slice(half * 512, half * 512 + 512)
                            pe.matmul(
                                g_pair[0:64, sl], lhsT=qwlt0[:, sc, :], rhs=p0[:, sl],
                                start=(sc == 0), stop=(sc == 7),
                                tile_position=(0, 0),
                            )
                            pe.matmul(
                                g_pair[64:128, sl], lhsT=qwlt1[:, sc, :], rhs=p1[:, sl],
                                start=(sc == 0), stop=(sc == 7),
                                tile_position=(0, 64),
                            )
                            pe.matmul(
                                rs_ps[64:72, sl], lhsT=eyes_sb[:, h0, :], rhs=p0[:, sl],
                                start=(px == 0 and sc == 0), stop=False,
                                tile_position=(0, 64),
                            )
                            pe.matmul(
                                rs_ps[64:72, sl], lhsT=eyes_sb[:, h1, :], rhs=p1[:, sl],
                                start=False, stop=(px == 3 and sc == 7),
                                tile_position=(0, 64),
                            )
                            pe.matmul(
                                rs_ps[96:104, sl], lhsT=eyes_sb[:, h0, :], rhs=p20[:, sl],
                                start=(px == 0 and sc == 0), stop=False,
                                tile_position=(0, 96),
                            )
                            pe.matmul(
                                rs_ps[96:104, sl], lhsT=eyes_sb[:, h1, :], rhs=p21[:, sl],
                                start=False, stop=(px == 3 and sc == 7),
                                tile_position=(0, 96),
                            )
                    act.activation(g_all[:, px, :], g_pair[:], AF.Copy)

                # ---------- batch stats (unchanged from v2)
                rs_stage = stats.tile([104, N], dt, tag="rs_stage", name=f"rsst_{b}")
                vec.tensor_copy(rs_stage[64:72, :], rs_ps[64:72, :])
                vec.tensor_copy(rs_stage[96:104, :], rs_ps[96:104, :])
                r_b = stats.tile([HEADS, N], dt, tag="r_b", name=f"rb_{b}")
                nc.sync.dma_start(r_b[:], rs_stage[64:72, :])
                ssq_sb = stats.tile([HEADS, N], dt, tag="ssq_sb", name=f"ssqsb_{b}")
                nc.sync.dma_start(ssq_sb[:], rs_stage[96:104, :])
                rinv = stats.tile([HEADS, N], dt, tag="rinv", name=f"rinv_{b}")
                vec.reciprocal(rinv[:], r_b[:])
                rinvsq = stats.tile([HEADS, N], dt, tag="rinvsq", name=f"rinvsq_{b}")
                vec.tensor_tensor(rinvsq[:], rinv[:], rinv[:], ALU.mult)
                ttr_scr = stats.tile([HEADS, N], dt, tag="ttr", name=f"ttr_{b}")
                vec.tensor_tensor(ttr_scr[:], ssq_sb[:], rinvsq[:], ALU.mult)
                s2 = stats.tile([HEADS, 1], dt, tag="s2", name=f"s2_{b}")
                vec.reduce_sum(s2[:], ttr_scr[:], axis=mybir.AxisListType.X)
                var = stats.tile([HEADS, 1], dt, tag="var", name=f"var_{b}")
                vec.tensor_scalar(
                    out=var[:], in0=s2[:], scalar1=1.0 / (float(N) * float(N)),
                    scalar2=-MU * MU, op0=ALU.mult, op1=ALU.add,
                )
                lnv = stats.tile([HEADS, 1], dt, tag="lnv", name=f"lnv_{b}")
                act.activation(lnv[:], var[:], AF.Ln, bias=cb_sb[0:HEADS, 1:2], scale=1.0)
                istd = stats.tile([HEADS, 1], dt, tag="istd", name=f"istd_{b}")
                act.activation(istd[:], lnv[:], AF.Exp, bias=cb_sb[0:HEADS, 2:3], scale=-0.5)
                c_b = stats.tile([HEADS, N], dt, tag="c_b", name=f"cb_{b}")
                vec.tensor_scalar(
                    out=c_b[:], in0=rinv[:], scalar1=istd[:], scalar2=None, op0=ALU.mult
                )
                istd_t = stats.tile([1, HEADS], dt, tag="istd_t", name=f"istdt_{b}")
                for h in range(HEADS):
                    nc.sync.dma_start(istd_t[0:1, h : h + 1], istd[h : h + 1, 0:1])
                ibc_ps = psum.tile([DH, HEADS], dt, tag="g", bufs=1, name=f"ibcps_{b}")
                pe.matmul(ibc_ps[:], lhsT=oner_sb[0:1, 0:DH], rhs=istd_t[:], start=True, stop=True)
                istd_bc = stats.tile([DH, HEADS], dt, tag="istd_bc", name=f"istdbc_{b}")
                act.activation(istd_bc[:], ibc_ps[:], AF.Copy)
                beta_t = stats.tile([DH, HEADS], dt, tag="beta_t", name=f"betat_{b}")
                vec.scalar_tensor_tensor(
                    out=beta_t[:], in0=wq_sb[:], scalar=-MU, in1=istd_bc[:],
                    op0=ALU.mult, op1=ALU.mult,
                )
                beta_sb = stats.tile([DH, HEADS], dt, tag="beta", name=f"beta_{b}")
                vec.tensor_scalar(
                    out=beta_sb[:], in0=beta_t[:], scalar1=blin2_sb[0:DH, :], scalar2=None,
                    op0=ALU.add,
                )

                # ---------- Phase B: pair-wide tail (no W_lin matmul needed)
                for px in range(4):
                    h = 2 * px
                    c_row = tails.tile([2, N], dt, tag="c_row", name=f"crow_{b}_{px}")
                    nc.sync.dma_start(c_row[0:1, :], c_b[h : h + 1, :])
                    nc.sync.dma_start(c_row[1:2, :], c_b[h + 1 : h + 2, :])
                    cbc_ps = psum.tile([128, N], dt, tag="et", bufs=2, name=f"cbcps_{b}_{px}")
                    for half in range(2):
                        sl = slice(half * 512, half * 512 + 512)
                        pe.matmul(
                            cbc_ps[:, sl], lhsT=sel2_sb[:], rhs=c_row[:, sl],
                            start=True, stop=True,
                        )
                    beta_pair = tails.tile([128, 1], dt, tag="beta_pair", name=f"bp_{b}_{px}")
                    nc.sync.dma_start(beta_pair[0:64, :], beta_sb[:, h : h + 1])
                    nc.sync.dma_start(beta_pair[64:128, :], beta_sb[:, h + 1 : h + 2])
                    xres = tails.tile([128, N], dt, tag="xres", name=f"xres_{b}_{px}")
                    nc.sync.dma_start(xres[:], xrd[b, 128 * px : 128 * px + 128, :])
                    t1 = tails.tile([128, N], dt, tag="t1", name=f"t1_{b}_{px}")
                    vec.tensor_tensor(t1[:], g_all[:, px, :], cbc_ps[:], ALU.mult)
                    f_sb = tails.tile([128, N], dt, tag="f", name=f"f_{b}_{px}")
                    vec.scalar_tensor_tensor(
                        out=f_sb[:], in0=t1[:], scalar=beta_pair[:],
                        in1=xres[:], op0=ALU.add, op1=ALU.add,
                    )
                    nc.sync.dma_start(outd[b, 128 * px : 128 * px + 128, :], f_sb[:])

    nc.compile()
    return nc


def host_inputs(x, W_start, b_start, rel_h, rel_w, W_lin, b_lin):
    x = np.asarray(x, np.float32)
    W_start = np.asarray(W_start, np.float32)
    b_start = np.asarray(b_start, np.float32)
    pos = (np.asarray(rel_h, np.float32) + np.asarray(rel_w, np.float32)).reshape(
        HEADS, DH, N
    )
    W_lin = np.asarray(W_lin, np.float32)
    b_lin = np.asarray(b_lin, np.float32)
    import ml_dtypes

    bf = ml_dtypes.bfloat16
    posd = np.ascontiguousarray((pos * SQ).reshape(C, N).astype(np.float32))
    bc = (b_start / SQ).reshape(4, 128).T.astype(np.float32)
    qc = bc * float(N)
    wlT = np.concatenate([(W_lin * SQ).T, (W_lin * SQ).T], axis=0)
    consts = {
        "wco": np.ascontiguousarray((W_start.T / SQ).astype(bf)),
        "posd": posd,
        "bc128": np.ascontiguousarray(bc),
        "qcorr": np.ascontiguousarray(qc.astype(np.float32)),
        "wl": np.ascontiguousarray(wlT.astype(np.float32)),
        "wlb": np.ascontiguousarray(wlT.astype(bf)),
        "blin2": np.ascontiguousarray(np.tile(b_lin, 2)[:, None].astype(np.float32)),
        "eyesd": np.ascontiguousarray(
            np.broadcast_to(np.eye(HEADS, dtype=np.float32), (128, HEADS, HEADS))
        ).astype(bf),
        "oner": np.ones((1, 128), np.float32),
        "sel2": np.ascontiguousarray(
            np.stack([
                np.concatenate([np.ones(64), np.zeros(64)]),
                np.concatenate([np.zeros(64), np.ones(64)]),
            ]).astype(np.float32)
        ),
        "cbias": np.ascontiguousarray(
            np.broadcast_to(
                np.array([-EBIAS, EPS, 0.0] + [0.0] * BUILD_SALT, np.float32),
                (128, 3 + BUILD_SALT),
            )
        ),
    }
    xr = x.reshape(B, C, N)
    in_maps = []
    for c in range(NCORES):
        m = dict(consts)
        m["xin"] = np.ascontiguousarray(xr[c * BPC : (c + 1) * BPC].astype(bf))
        m["xrd"] = np.ascontiguousarray(xr[c * BPC : (c + 1) * BPC])
        in_maps.append(m)
    return in_maps


_PROG = None


def kernel(**inputs):
    global _PROG
    if _PROG is None:
        _PROG = build_program()
    in_maps = host_inputs(**inputs)
    res = bass_utils.run_bass_kernel_spmd(_PROG, in_maps, core_ids=list(range(NCORES)))
    out = np.concatenate([r["outd"] for r in res.results], axis=0)
    return out.reshape(B, C, 32, 32)


# revision 5
# speedup vs baseline: 1.1000x; 1.0028x over previous
"""CMHSA Trainium2 kernel v3 (nn_CMHSA_56487409877161).

v3 structure per core (4 batches):
  startconv fwd only (splain, spos; spos emits qsum via accum_out).
  Per (b,h): qwlt = splain_h^T @ W_lin'  (8 J=64 matmuls, [128,8,64] bf16),
  ET matmuls (softmax axis on partitions), exp on ACT, square on DVE,
  then per chunk: H (=W_lin G fused, lhsT=qwlt chunk), r/ssq eyes streams.
  Batch stats as in v2. Phase B: cbc pair broadcast (2 K=1 matmuls into one
  [128,N] psum), t1 = H_all*cbc (PSUM operand), f = t1 + beta + x, pair-wide.
"""

import numpy as np

import concourse.bass as bass
import concourse.mybir as mybir
import concourse.tile as tile
from concourse import bacc, bass_utils

B, C, N = 32, 512, 1024
HEADS, DH = 8, 64
NCORES = 8
BPC = B // NCORES
EPS = 1e-5
SCALE = (C / 4.0) ** 0.5
SQ = float(np.sqrt(SCALE))
EBIAS = 45.0
MU = 1.0 / N
BUILD_SALT = 38

F32 = mybir.dt.float32
BF16 = mybir.dt.bfloat16
AF = mybir.ActivationFunctionType
ALU = mybir.AluOpType

MMDT = BF16


def build_program():
    nc = bacc.Bacc("TRN2", target_bir_lowering=False)
    dt = F32
    pdt = BF16
    xin = nc.dram_tensor("xin", [BPC, C, N], MMDT, kind="ExternalInput").ap()
    xrd = nc.dram_tensor("xrd", [BPC, C, N], F32, kind="ExternalInput").ap()
    wco = nc.dram_tensor("wco", [C, C], MMDT, kind="ExternalInput").ap()
    posd = nc.dram_tensor("posd", [C, N], dt, kind="ExternalInput").ap()
    bc128 = nc.dram_tensor("bc128", [128, 4], dt, kind="ExternalInput").ap()
    qcorr = nc.dram_tensor("qcorr", [128, 4], dt, kind="ExternalInput").ap()
    wl = nc.dram_tensor("wl", [128, DH], F32, kind="ExternalInput").ap()
    wlb = nc.dram_tensor("wlb", [128, DH], MMDT, kind="ExternalInput").ap()
    blin2 = nc.dram_tensor("blin2", [128, 1], dt, kind="ExternalInput").ap()
    eyesd = nc.dram_tensor("eyesd", [128, HEADS, HEADS], pdt, kind="ExternalInput").ap()
    oner = nc.dram_tensor("oner", [1, 128], F32, kind="ExternalInput").ap()
    sel2 = nc.dram_tensor("sel2", [2, 128], F32, kind="ExternalInput").ap()
    cbias = nc.dram_tensor("cbias", [128, 3 + BUILD_SALT], dt, kind="ExternalInput").ap()
    outd = nc.dram_tensor("outd", [BPC, C, N], dt, kind="ExternalOutput").ap()

    act = nc.scalar
    vec = nc.vector
    pe = nc.tensor

    with tile.TileContext(nc) as tc:
        with (
            tc.tile_pool(name="consts", bufs=1) as consts,
            tc.tile_pool(name="xpool", bufs=1) as xpool,
            tc.tile_pool(name="spool", bufs=1) as spool,
            tc.tile_pool(name="gpool", bufs=1) as gpool,
            tc.tile_pool(name="qwpool", bufs=4) as qwpool,
            tc.tile_pool(name="ppool", bufs=6) as ppool,
            tc.tile_pool(name="p2pool", bufs=4) as p2pool,
            tc.tile_pool(name="tails", bufs=3) as tails,
            tc.tile_pool(name="stats", bufs=1) as stats,
            tc.tile_pool(name="psum", bufs=1, space="PSUM") as psum,
        ):
            wco_sb = consts.tile([128, 4, C], MMDT)
            nc.sync.dma_start(wco_sb[:], wco.rearrange("(cc p) o -> p cc o", p=128))
            pos_sb = consts.tile([128, 4, N], dt)
            nc.sync.dma_start(pos_sb[:], posd.rearrange("(cc p) n -> p cc n", p=128))
            bc128_sb = consts.tile([128, 4], dt)
            nc.sync.dma_start(bc128_sb[:], bc128)
            qcorr_sb = consts.tile([128, 4], dt)
            nc.sync.dma_start(qcorr_sb[:], qcorr)
            wl_sb = consts.tile([128, DH], F32)
            nc.sync.dma_start(wl_sb[:], wl)
            wlb_sb = consts.tile([128, DH], MMDT)
            nc.sync.dma_start(wlb_sb[:], wlb)
            blin2_sb = consts.tile([128, 1], dt)
            nc.sync.dma_start(blin2_sb[:], blin2)
            eyes_sb = consts.tile([128, HEADS, HEADS], pdt)
            nc.sync.dma_start(eyes_sb[:], eyesd)
            oner_sb = consts.tile([1, 128], F32)
            nc.sync.dma_start(oner_sb[:], oner)
            sel2_sb = consts.tile([2, 128], F32)
            nc.sync.dma_start(sel2_sb[:], sel2)
            cb_sb = consts.tile([128, 3], dt)
            nc.sync.dma_start(cb_sb[:], cbias[:, 0:3])
            zeros_sb = consts.tile([128, N], dt)
            nc.vector.memset(zeros_sb[:], 0.0)

            for b in range(BPC):
                # ---------- startconv forward
                x_sb = xpool.tile([128, 4, N], MMDT, tag="x", name=f"x_{b}")
                nc.sync.dma_start(x_sb[:], xin[b].rearrange("(cc p) n -> p cc n", p=128))

                splain = spool.tile([128, 4, N], MMDT, tag="splain", name=f"splain_{b}")
                spos = spool.tile([128, 4, N], MMDT, tag="spos", name=f"spos_{b}")
                qs_raw = stats.tile([128, 4], dt, tag="qs_raw", name=f"qsraw_{b}")

                for pc in range(4):
                    s_ps = psum.tile([128, N], dt, tag="et", bufs=2, name=f"sps_{b}_{pc}")
                    for cc in range(4):
                        for half in range(2):
                            pe.matmul(
                                s_ps[:, half * 512 : half * 512 + 512],
                                lhsT=wco_sb[:, cc, 128 * pc : 128 * pc + 128],
                                rhs=x_sb[:, cc, half * 512 : half * 512 + 512],
                                start=(cc == 0),
                                stop=(cc == 3),
                            )
                    vec.scalar_tensor_tensor(
                        out=splain[:, pc, :], in0=s_ps[:],
                        scalar=bc128_sb[:, pc : pc + 1],
                        in1=zeros_sb[:], op0=ALU.add, op1=ALU.add,
                        accum_out=qs_raw[:, pc : pc + 1],
                    )
                    vec.scalar_tensor_tensor(
                        out=spos[:, pc, :], in0=s_ps[:],
                        scalar=bc128_sb[:, pc : pc + 1],
                        in1=pos_sb[:, pc, :], op0=ALU.add, op1=ALU.add,
                    )

                # qs_used = qs_raw - qcorr  (= qsum/SQ in channel layout)
                qs_used = stats.tile([128, 4], dt, tag="qs_used", name=f"qsu_{b}")
                vec.tensor_tensor(qs_used[:], qs_raw[:], qcorr_sb[:], ALU.subtract)

                # wq[e, h] = sum_d wl[d, e] * qs_used[d_h]  (per head)
                wq_ps = psum.tile([DH, HEADS], dt, tag="g", bufs=1, name=f"wqps_{b}")
                for h in range(HEADS):
                    prow = (h % 2) * 64
                    pcix = h // 2
                    pe.matmul(
                        wq_ps[:, h : h + 1],
                        lhsT=wl_sb[prow : prow + 64, :],
                        rhs=qs_used[prow : prow + 64, pcix : pcix + 1],
                        start=True, stop=True,
                        tile_position=(prow, 0),
                    )
                wq_sb = stats.tile([DH, HEADS], dt, tag="wq", name=f"wq_{b}")
                vec.tensor_copy(wq_sb[:], wq_ps[:])

                # ---------- per-head maps
                rs_ps = psum.tile([104, N], dt, tag="rs", name=f"rsps_{b}")
                g_all = gpool.tile([128, 4, N], F32, tag="g_all", name=f"gall_{b}")

                for px in range(4):
                    h0, h1 = 2 * px, 2 * px + 1
                    # qwlt[n, e] = sum_d splain_h[d, n] * wlb[d, e]  (both heads)
                    qw_ps = psum.tile([128, 8 * DH], dt, tag="et", bufs=2, name=f"qwps_{b}_{h0}")
                    qw_ps2 = psum.tile([128, 8 * DH], dt, tag="et", bufs=2, name=f"qwps_{b}_{h1}")
                    for sc in range(8):
                        pe.matmul(
                            qw_ps[:, DH * sc : DH * sc + DH],
                            lhsT=splain[0:64, px, 128 * sc : 128 * sc + 128],
                            rhs=wlb_sb[0:64, :],
                            start=True, stop=True,
                            tile_position=(0, 0),
                        )
                        pe.matmul(
                            qw_ps2[:, DH * sc : DH * sc + DH],
                            lhsT=splain[64:128, px, 128 * sc : 128 * sc + 128],
                            rhs=wlb_sb[64:128, :],
                            start=True, stop=True,
                            tile_position=(64, 0),
                        )
                    qwlt0 = qwpool.tile([128, 8, DH], MMDT, tag="qwlt", name=f"qwlt_{b}_{h0}")
                    vec.tensor_copy(qwlt0[:], qw_ps[:])
                    qwlt1 = qwpool.tile([128, 8, DH], MMDT, tag="qwlt", name=f"qwlt_{b}_{h1}")
                    vec.tensor_copy(qwlt1[:], qw_ps2[:])

                    g_pair = psum.tile([128, N], dt, tag="g", bufs=1, name=f"gps_{b}_{px}")
                    for sc in range(8):
                        et0 = psum.tile([128, N], dt, tag="et", bufs=2, name=f"et_{b}_{h0}_{sc}")
                        et1 = psum.tile([128, N], dt, tag="et", bufs=2, name=f"et_{b}_{h1}_{sc}")
                        for half in range(2):
                            sl = slice(half * 512, half * 512 + 512)
                            pe.matmul(
                                et0[:, sl],
                                lhsT=splain[0:64, px, 128 * sc : 128 * sc + 128],
                                rhs=spos[0:64, px, sl],
                                start=True, stop=True,
                                tile_position=(0, 0),
                            )
                            pe.matmul(
                                et1[:, sl],
                                lhsT=splain[64:128, px, 128 * sc : 128 * sc + 128],
                                rhs=spos[64:128, px, sl],
                                start=True, stop=True,
                                tile_position=(64, 0),
                            )
                        p0 = ppool.tile([128, N], pdt, tag="p", name=f"p_{b}_{h0}_{sc}")
                        act.activation(p0[:], et0[:], AF.Exp, bias=cb_sb[:, 0:1], scale=1.0)
                        p1 = ppool.tile([128, N], pdt, tag="p", name=f"p_{b}_{h1}_{sc}")
                        act.activation(p1[:], et1[:], AF.Exp, bias=cb_sb[:, 0:1], scale=1.0)
                        p20 = p2pool.tile([128, N], pdt, tag="p2", name=f"p2_{b}_{h0}_{sc}")
                        vec.tensor_tensor(p20[:], p0[:], p0[:], ALU.mult)
                        p21 = p2pool.tile([128, N], pdt, tag="p2", name=f"p2_{b}_{h1}_{sc}")
                        vec.tensor_tensor(p21[:], p1[:], p1[:], ALU.mult)
                        for half in range(2):
                            sl = slice(half * 512, half * 512 + 512)
                            pe.matmul(
                                g_pair[0:64, sl], lhsT=qwlt0[:, sc, :], rhs=p0[:, sl],
                                start=(sc == 0), stop=(sc == 7),
                                tile_position=(0, 0),
                            )
                            pe.matmul(
                                g_pair[64:128, sl], lhsT=qwlt1[:, sc, :], rhs=p1[:, sl],
                                start=(sc == 0), stop=(sc == 7),
                                tile_position=(0, 64),
                            )
                            pe.matmul(
                                rs_ps[64:72, sl], lhsT=eyes_sb[:, h0, :], rhs=p0[:, sl],
                                start=(px == 0 and sc == 0), stop=False,
                                tile_position=(0, 64),
                            )
                            pe.matmul(
                                rs_ps[64:72, sl], lhsT=eyes_sb[:, h1, :], rhs=p1[:, sl],
                                start=False, stop=(px == 3 and sc == 7),
                                tile_position=(0, 64),
                            )
                            pe.matmul(
                                rs_ps[96:104, sl], lhsT=eyes_sb[:, h0, :], rhs=p20[:, sl],
                                start=(px == 0 and sc == 0), stop=False,
                                tile_position=(0, 96),
                            )
                            pe.matmul(
                                rs_ps[96:104, sl], lhsT=eyes_sb[:, h1, :], rhs=p21[:, sl],
                                start=False, stop=(px == 3 and sc == 7),
                                tile_position=(0, 96),
                            )
                    act.activation(g_all[:, px, :], g_pair[:], AF.Copy)

                # ---------- batch stats (unchanged from v2)
                rs_stage = stats.tile([104, N], dt, tag="rs_stage", name=f"rsst_{b}")
                vec.tensor_copy(rs_stage[64:72, :], rs_ps[64:72, :])
                vec.tensor_copy(rs_stage[96:104, :], rs_ps[96:104, :])
                r_b = stats.tile([HEADS, N], dt, tag="r_b", name=f"rb_{b}")
                nc.sync.dma_start(r_b[:], rs_stage[64:72, :])
                ssq_sb = stats.tile([HEADS, N], dt, tag="ssq_sb", name=f"ssqsb_{b}")
                nc.sync.dma_start(ssq_sb[:], rs_stage[96:104, :])
                rinv = stats.tile([HEADS, N], dt, tag="rinv", name=f"rinv_{b}")
                vec.reciprocal(rinv[:], r_b[:])
                rinvsq = stats.tile([HEADS, N], dt, tag="rinvsq", name=f"rinvsq_{b}")
                vec.tensor_tensor(rinvsq[:], rinv[:], rinv[:], ALU.mult)
                ttr_scr = stats.tile([HEADS, N], dt, tag="ttr", name=f"ttr_{b}")
                vec.tensor_tensor(ttr_scr[:], ssq_sb[:], rinvsq[:], ALU.mult)
                s2 = stats.tile([HEADS, 1], dt, tag="s2", name=f"s2_{b}")
                vec.reduce_sum(s2[:], ttr_scr[:], axis=mybir.AxisListType.X)
                var = stats.tile([HEADS, 1], dt, tag="var", name=f"var_{b}")
                vec.tensor_scalar(
                    out=var[:], in0=s2[:], scalar1=1.0 / (float(N) * float(N)),
                    scalar2=-MU * MU, op0=ALU.mult, op1=ALU.add,
                )
                lnv = stats.tile([HEADS, 1], dt, tag="lnv", name=f"lnv_{b}")
                act.activation(lnv[:], var[:], AF.Ln, bias=cb_sb[0:HEADS, 1:2], scale=1.0)
                istd = stats.tile([HEADS, 1], dt, tag="istd", name=f"istd_{b}")
                act.activation(istd[:], lnv[:], AF.Exp, bias=cb_sb[0:HEADS, 2:3], scale=-0.5)
                c_b = stats.tile([HEADS, N], dt, tag="c_b", name=f"cb_{b}")
                vec.tensor_scalar(
                    out=c_b[:], in0=rinv[:], scalar1=istd[:], scalar2=None, op0=ALU.mult
                )
                istd_t = stats.tile([1, HEADS], dt, tag="istd_t", name=f"istdt_{b}")
                for h in range(HEADS):
                    nc.sync.dma_start(istd_t[0:1, h : h + 1], istd[h : h + 1, 0:1])
                ibc_ps = psum.tile([DH, HEADS], dt, tag="g", bufs=1, name=f"ibcps_{b}")
                pe.matmul(ibc_ps[:], lhsT=oner_sb[0:1, 0:DH], rhs=istd_t[:], start=True, stop=True)
                istd_bc = stats.tile([DH, HEADS], dt, tag="istd_bc", name=f"istdbc_{b}")
                act.activation(istd_bc[:], ibc_ps[:], AF.Copy)
                beta_t = stats.tile([DH, HEADS], dt, tag="beta_t", name=f"betat_{b}")
                vec.scalar_tensor_tensor(
                    out=beta_t[:], in0=wq_sb[:], scalar=-MU, in1=istd_bc[:],
                    op0=ALU.mult, op1=ALU.mult,
                )
                beta_sb = stats.tile([DH, HEADS], dt, tag="beta", name=f"beta_{b}")
                vec.tensor_scalar(
                    out=beta_sb[:], in0=beta_t[:], scalar1=blin2_sb[0:DH, :], scalar2=None,
                    op0=ALU.add,
                )

                # ---------- Phase B: pair-wide tail (no W_lin matmul needed)
                for px in range(4):
                    h = 2 * px
                    c_row = tails.tile([2, N], dt, tag="c_row", name=f"crow_{b}_{px}")
                    nc.sync.dma_start(c_row[0:1, :], c_b[h : h + 1, :])
                    nc.sync.dma_start(c_row[1:2, :], c_b[h + 1 : h + 2, :])
                    cbc_ps = psum.tile([128, N], dt, tag="et", bufs=2, name=f"cbcps_{b}_{px}")
                    for half in range(2):
                        sl = slice(half * 512, half * 512 + 512)
                        pe.matmul(
                            cbc_ps[:, sl], lhsT=sel2_sb[:], rhs=c_row[:, sl],
                            start=True, stop=True,
                        )
                    beta_pair = tails.tile([128, 1], dt, tag="beta_pair", name=f"bp_{b}_{px}")
                    nc.sync.dma_start(beta_pair[0:64, :], beta_sb[:, h : h + 1])
                    nc.sync.dma_start(beta_pair[64:128, :], beta_sb[:, h + 1 : h + 2])
                    xres = tails.tile([128, N], dt, tag="xres", name=f"xres_{b}_{px}")
                    nc.sync.dma_start(xres[:], xrd[b, 128 * px : 128 * px + 128, :])
                    t1 = tails.tile([128, N], dt, tag="t1", name=f"t1_{b}_{px}")
                    vec.tensor_tensor(t1[:], g_all[:, px, :], cbc_ps[:], ALU.mult)
                    f_sb = tails.tile([128, N], dt, tag="f", name=f"f_{b}_{px}")
                    vec.scalar_tensor_tensor(
                        out=f_sb[:], in0=t1[:], scalar=beta_pair[:],
                        in1=xres[:], op0=ALU.add, op1=ALU.add,
                    )
                    nc.sync.dma_start(outd[b, 128 * px : 128 * px + 128, :], f_sb[:])

    nc.compile()
    return nc


def host_inputs(x, W_start, b_start, rel_h, rel_w, W_lin, b_lin):
    x = np.asarray(x, np.float32)
    W_start = np.asarray(W_start, np.float32)
    b_start = np.asarray(b_start, np.float32)
    pos = (np.asarray(rel_h, np.float32) + np.asarray(rel_w, np.float32)).reshape(
        HEADS, DH, N
    )
    W_lin = np.asarray(W_lin, np.float32)
    b_lin = np.asarray(b_lin, np.float32)
    import ml_dtypes

    bf = ml_dtypes.bfloat16
    posd = np.ascontiguousarray((pos * SQ).reshape(C, N).astype(np.float32))
    bc = (b_start / SQ).reshape(4, 128).T.astype(np.float32)
    qc = bc * float(N)
    wlT = np.concatenate([(W_lin * SQ).T, (W_lin * SQ).T], axis=0)
    consts = {
        "wco": np.ascontiguousarray((W_start.T / SQ).astype(bf)),
        "posd": posd,
        "bc128": np.ascontiguousarray(bc),
        "qcorr": np.ascontiguousarray(qc.astype(np.float32)),
        "wl": np.ascontiguousarray(wlT.astype(np.float32)),
        "wlb": np.ascontiguousarray(wlT.astype(bf)),
        "blin2": np.ascontiguousarray(np.tile(b_lin, 2)[:, None].astype(np.float32)),
        "eyesd": np.ascontiguousarray(
            np.broadcast_to(np.eye(HEADS, dtype=np.float32), (128, HEADS, HEADS))
        ).astype(bf),
        "oner": np.ones((1, 128), np.float32),
        "sel2": np.ascontiguousarray(
            np.stack([
                np.concatenate([np.ones(64), np.zeros(64)]),
                np.concatenate([np.zeros(64), np.ones(64)]),
            ]).astype(np.float32)
        ),
        "cbias": np.ascontiguousarray(
            np.broadcast_to(
                np.array([-EBIAS, EPS, 0.0] + [0.0] * BUILD_SALT, np.float32),
                (128, 3 + BUILD_SALT),
            )
        ),
    }
    xr = x.reshape(B, C, N)
    in_maps = []
    for c in range(NCORES):
        m = dict(consts)
        m["xin"] = np.ascontiguousarray(xr[c * BPC : (c + 1) * BPC].astype(bf))
        m["xrd"] = np.ascontiguousarray(xr[c * BPC : (c + 1) * BPC])
        in_maps.append(m)
    return in_maps


_PROG = None


def kernel(**inputs):
    global _PROG
    if _PROG is None:
        _PROG = build_program()
    in_maps = host_inputs(**inputs)
    res = bass_utils.run_bass_kernel_spmd(_PROG, in_maps, core_ids=list(range(NCORES)))
    out = np.concatenate([r["outd"] for r in res.results], axis=0)
    return out.reshape(B, C, 32, 32)
